# revision 17
# baseline (speedup 1.0000x reference)
"""Trainium2 Bass kernel for GAT-style exercise->KC message passing (v2).

Math (per reference):
  kc_Wh = kc_h @ W1
  z[i,j] = ex_score[i] + kc_score[j]
  p[i,j] = adj * exp(leaky(z)) = adj * max(exp(z), C_i * D_j)
           C = exp(0.2 ex_score), D = exp(0.2 kc_score)
  attn   = p / rowsum(p)   (rowsum via ones column in the attention matmul)
  out    = elu((attn @ kc_Wh) * (exercise_h @ E)), elu(x)=max(x, min(exp(x),1)-1)

Layout: KC on partitions (8 j-chunks of 128), EX on the free axis in 7
stripes of 896 cols. Pipeline per (stripe, j) item:
  ACT exp(z) -> DVE ts (Cb*D_j) + tt max -> mask mult (GPS head / DVE tail)
  -> PE att matmuls per stripe -> DVE/ACT elu epilogue -> SP store.
Eh = exercise_h @ E is computed early by PE and drained to SBUF bf16.
All DRAM I/O bf16/int8; psum fp32. Exercise rows sharded 8 ways.

DMA-completion semaphores can increment out of order across queues, so
every DMA wait is against the FULL count of a dedicated semaphore (or a
parity/slot-split one where issue-order gating bounds the contributors).
"""

import sys

sys.path.insert(0, "/opt/trn_rl_repo")

import numpy as np

N_CORES = 8
N_EX = 50000
N_KC = 1024
D = 256
SHARD = N_EX // N_CORES          # 6250
PAD = 6272                       # 49 * 128
BLOCKS = PAD // 128              # 49
NS = 7                           # stripes
W = PAD // NS                    # 896
BPS = W // 128                   # 7 blocks per stripe
ALPHA = 0.2
WPK = 1808
CHUNK = 448
NCH = PAD // CHUNK               # 14
QW = PAD // 4                    # 1568 exT load quarter
HALVES = [(0, 3584), (3584, 2688)]   # stripe-aligned halves (0-3 / 4-6)
NG = (BLOCKS + 1) // 2           # 25 elu/store groups
MASK_GPS = 640                   # mask cols [0,MASK_GPS) on gpsimd
A_ITEMS = {(s, j) for s in range(7) for j in (1, 3, 5, 7)}   # Prelu+Exp path

_CACHE = {}


def _build_nc(dbg=()):
    import concourse.bass as bass
    import concourse.mybir as mybir

    f32 = mybir.dt.float32
    bf16 = mybir.dt.bfloat16
    i8 = mybir.dt.int8
    AF = mybir.ActivationFunctionType
    ALU = mybir.AluOpType

    nc = bass.Bass()

    exT_d = nc.declare_dram_parameter("exT", [D, PAD], bf16, isOutput=False)
    adj_d = nc.declare_dram_parameter("adjT", [N_KC, PAD], i8, isOutput=False)
    wpk_d = nc.declare_dram_parameter("wpack", [D, WPK], bf16, isOutput=False)
    e_d = nc.declare_dram_parameter("eMat", [D, D], bf16, isOutput=False)
    out_d = nc.declare_dram_parameter("out", [PAD, D], bf16, isOutput=True)
    exrow_s = nc.dram_tensor("exrow_s", [1, PAD], bf16)
    crow_s = nc.dram_tensor("crow_s", [1, PAD], bf16)

    from contextlib import ExitStack

    es = ExitStack()
    _ctr = [0]

    def _nm(pfx):
        _ctr[0] += 1
        return f"{pfx}{_ctr[0]}"

    sb = lambda shape, dt: es.enter_context(nc.sbuf_tensor(_nm("t"), shape, dt))
    ps = lambda shape, dt: es.enter_context(nc.psum_tensor(_nm("p"), shape, dt))
    sem = lambda: es.enter_context(nc.semaphore(name=_nm("s")))

    ITEMS = [(s, j) for s in range(NS) for j in range(8)]
    IDX = {it: k for k, it in enumerate(ITEMS)}
    BORD = {}
    nb = 0
    for it in ITEMS:
        if it not in A_ITEMS:
            nb += 1
            BORD[it] = nb
    # per-engine done-counts through item k (inclusive): B on DVE, A on ACT
    NBC, NAC = [], []
    cb_, ca_ = 0, 0
    for it in ITEMS:
        if it in A_ITEMS:
            ca_ += 1
        else:
            cb_ += 1
        NBC.append(cb_)
        NAC.append(ca_)

    ADJH = 3584   # adj ring buffer width; half 1 (2688) reloads at col 0

    def adj_col(s):
        return s * W if s < 4 else s * W - ADJH

    with es:
        # ---- SBUF ----
        wp0 = sb([128, WPK], bf16); wp1 = sb([128, WPK], bf16)
        ebf0 = sb([128, D], bf16); ebf1 = sb([128, D], bf16)
        a2b = sb([128, D], bf16)
        w1a1c = sb([128, 2], bf16)
        kcwhE_all = sb([128, 8 * 264], bf16)
        kc_score = sb([128, 8], f32)
        kcs_tmp = sb([128, D], f32)
        drow = sb([128, 8], f32)
        adj_all = sb([128, 8 * ADJH], bf16)
        exT0 = sb([128, PAD], bf16); exT1 = sb([128, PAD], bf16)
        exb = sb([128, PAD], bf16)
        cb = sb([128, PAD], bf16)
        pm2 = sb([128, 2 * 8 * W], bf16)
        ltb = sb([128, W], bf16)
        excp = sb([1, 2 * CHUNK], bf16)
        crow128 = sb([128, 28], bf16)
        crowE = sb([128, 28], bf16)
        ehb = sb([128, BLOCKS * D], bf16)
        dcol = sb([128, 2 * BPS], f32)
        recipb = sb([128, 2 * BPS], f32)
        zb = sb([128, 4 * 512], bf16)
        ebuf = sb([128, 4 * 512], bf16)
        mb = sb([128, 4 * 512], bf16)

        # ---- PSUM (16 KB/partition) ----
        ps_att = ps([128, BPS * 512], f32)

        def eh_slot(b):
            lo = 512 * (3 + b % 4)
            return ps_att[:, lo : lo + D]

        ps_w1a1 = ps_att[:, 1472:1474]
        ps_exsc = [
            ps_att[0:1, 0:CHUNK],
            ps_att[0:1, 512 : 512 + CHUNK],
            ps_att[0:1, 1024 : 1024 + CHUNK],
        ]
        ps_kcwh = [ps_att[:, 1536:1792], ps_att[:, 2048:2304]]

        wp = [wp0, wp1]
        exT = [exT0, exT1]
        ebf = [ebf0, ebf1]
        w1 = [wp[t][:, 0:D] for t in range(2)]
        w1T = [wp[t][:, D : 2 * D] for t in range(2)]
        kchT = [wp[t][:, 2 * D : 2 * D + N_KC] for t in range(2)]
        a1col = [wp[t][:, 1536:1537] for t in range(2)]
        kcwhE = [kcwhE_all[:, 264 * j : 264 * j + 258] for j in range(8)]
        adjb = [adj_all[:, ADJH * j : ADJH * (j + 1)] for j in range(8)]

        def pmv(s, j):
            base = (s % 2) * 8 * W + j * W
            return pm2[:, base : base + W]

        s_wp = sem(); s_ebf = sem(); s_a2b = sem()
        s_exq = [sem() for _ in range(4)]
        s_adjt = [sem() for _ in range(8)]
        s_w1a1 = sem(); s_w1a1c = sem(); s_kcwh = sem(); s_kcj = sem()
        s_dj = sem(); s_exsc = sem(); s_excp = sem()
        s_bnc = [sem(), sem()]           # bounce stores by chunk parity
        s_exb = [sem(), sem()]
        s_crl = [sem(), sem()]
        s_cre = sem()
        s_crs = [sem(), sem()]
        s_cb = [sem(), sem()]
        s_pmB = sem(); s_pmA = sem(); s_t2 = sem()
        s_maskG = sem(); s_maskD = sem()
        s_attmm = sem(); s_ehmm = sem(); s_ehcp = sem()
        s_stt = sem(); s_eluE = sem(); s_mb = sem()
        s_st = [sem() for _ in range(4)]   # out stores by group%4
        s_fin = sem(); s_rc = sem(); s_lt = sem()

        block = es.enter_context(nc.Block())

        # ---------------- SYNC (SP): HWDGE plain DMAs ----------------
        @block.sync
        def _(sync):
            sync.dma_start(out=wp0[:, :], in_=wpk_d[0:128, :]).then_inc(s_wp, 16)
            sync.dma_start(out=wp1[:, :], in_=wpk_d[128:256, :]).then_inc(s_wp, 16)
            sync.dma_start(out=ebf0[:, :], in_=e_d[0:128, :]).then_inc(s_ebf, 16)
            sync.dma_start(out=ebf1[:, :], in_=e_d[128:256, :]).then_inc(s_ebf, 16)
            sync.dma_start(
                out=a2b[:, :],
                in_=wpk_d[0:1, 1537 : 1537 + D].to_broadcast((128, D)),
            ).then_inc(s_a2b, 16)
            for q in range(4):
                for t in range(2):
                    sync.dma_start(
                        out=exT[t][:, q * QW : (q + 1) * QW],
                        in_=exT_d[128 * t : 128 * (t + 1), q * QW : (q + 1) * QW],
                    ).then_inc(s_exq[q], 16)
            for c in range(NCH):
                sync.wait_ge(s_excp, c + 1)
                sync.dma_start(
                    out=exrow_s[0:1, c * CHUNK : (c + 1) * CHUNK],
                    in_=excp[0:1, (c % 2) * CHUNK : (c % 2) * CHUNK + CHUNK],
                ).then_inc(s_bnc[c % 2], 16)
            for h in range(2):
                off, hw = HALVES[h]
                kw = hw // 128
                nche = (off + hw) // CHUNK   # 8 / 14 chunks total
                sync.wait_ge(s_bnc[0], 16 * ((nche + 1) // 2))
                sync.wait_ge(s_bnc[1], 16 * (nche // 2))
                sync.dma_start(
                    out=exb[:, off : off + hw],
                    in_=exrow_s[0:1, off : off + hw].to_broadcast((128, hw)),
                ).then_inc(s_exb[h], 16)
                if h == 1:
                    sync.wait_ge(s_cre, 1)   # crow128 reuse (h0 exp done)
                sync.dma_start(
                    out=crow128[:, 0:kw], in_=exrow_s[0:1, off : off + hw]
                ).then_inc(s_crl[h], 16)
                sync.wait_ge(s_cre, h + 1)
                sync.dma_start(
                    out=crow_s[0:1, off : off + hw], in_=crowE[:, 0:kw]
                ).then_inc(s_crs[h], 16)
                sync.wait_ge(s_crs[h], 16)
                sync.dma_start(
                    out=cb[:, off : off + hw],
                    in_=crow_s[0:1, off : off + hw].to_broadcast((128, hw)),
                ).then_inc(s_cb[h], 16)
            stq = [0, 0, 0, 0]
            for g in range(NG):
                gw = 512 if 2 * g + 1 < BLOCKS else 256
                sync.wait_ge(s_mb, g + 1)
                if gw == 512:
                    sync.dma_start(
                        out=out_d[256 * g : 256 * g + 256, :].rearrange(
                            "(q p) c -> p q c", p=128
                        ),
                        in_=mb[:, (g % 4) * 512 : (g % 4) * 512 + 512],
                    ).then_inc(s_st[g % 4], 16)
                else:
                    sync.dma_start(
                        out=out_d[256 * g : 256 * g + 128, :],
                        in_=mb[:, (g % 4) * 512 : (g % 4) * 512 + 256],
                    ).then_inc(s_st[g % 4], 16)
                stq[g % 4] += 1
            for q in range(4):
                sync.wait_ge(s_st[q], 16 * stq[q])

        # ---------------- GPSIMD: SWDGE cast DMAs + mask head ----------------
        @block.gpsimd
        def _(gp):
            for j in range(8):
                gp.dma_start(
                    out=adjb[j][:, 0:ADJH],
                    in_=adj_d[128 * j : 128 * (j + 1), 0:ADJH],
                ).then_inc(s_adjt[j], 16)
            c = MASK_GPS
            for k, (s, j) in enumerate(ITEMS):
                gp.wait_ge(s_pmB, NBC[k])
                if NAC[k]:
                    gp.wait_ge(s_pmA, NAC[k])
                gp.wait_ge(s_adjt[j], 16 if s < 4 else 32)
                lo = adj_col(s)
                gp.tensor_tensor(
                    out=pmv(s, j)[:, 0:c],
                    in0=pmv(s, j)[:, 0:c],
                    in1=adjb[j][:, lo : lo + c],
                    op=ALU.mult,
                ).then_inc(s_maskG, 1)
                if s == 3:
                    gp.wait_ge(s_maskD, IDX[(3, j)] + 1)
                    gp.wait_ge(s_maskG, IDX[(3, j)] + 1)   # own head retired
                    gp.dma_start(
                        out=adjb[j][:, 0 : HALVES[1][1]],
                        in_=adj_d[
                            128 * j : 128 * (j + 1),
                            HALVES[1][0] : HALVES[1][0] + HALVES[1][1],
                        ],
                    ).then_inc(s_adjt[j], 16)

        # ---------------- PE: all matmuls ----------------
        @block.tensor
        def _(pe):
            pe.wait_ge(s_wp, 32)
            for t in range(2):
                for kt in range(2):
                    mm = nc.tensor.matmul(
                        ps_w1a1[:, t : t + 1],
                        w1T[kt][:, 128 * t : 128 * (t + 1)],
                        a1col[kt],
                        start=(kt == 0),
                        stop=(kt == 1),
                    )
                    if t == 1 and kt == 1:
                        mm.then_inc(s_w1a1, 1)
            for j in range(8):
                if j >= 2:
                    pe.wait_ge(s_kcj, j - 1)
                for t in range(2):
                    mm = nc.tensor.matmul(
                        ps_kcwh[j % 2],
                        kchT[t][:, 128 * j : 128 * (j + 1)],
                        w1[t],
                        start=(t == 0),
                        stop=(t == 1),
                    )
                    if t == 1:
                        mm.then_inc(s_kcwh, 1)
            pe.wait_ge(s_w1a1c, 1)
            qdone = -1
            for cix in range(NCH):
                qe = ((cix + 1) * CHUNK - 1) // QW
                while qdone < qe:
                    qdone += 1
                    pe.wait_ge(s_exq[qdone], 32)
                if cix >= 3:
                    pe.wait_ge(s_excp, cix - 2)
                for t in range(2):
                    mm = nc.tensor.matmul(
                        ps_exsc[cix % 3],
                        w1a1c[:, t : t + 1],
                        exT[t][:, cix * CHUNK : (cix + 1) * CHUNK],
                        start=(t == 0),
                        stop=(t == 1),
                    )
                    if t == 1:
                        mm.then_inc(s_exsc, 1)
            # Eh for all blocks (early), rotating through ps_att banks 3-6
            pe.wait_ge(s_ebf, 32)
            pe.wait_ge(s_kcj, 8)   # kcwh drains done (banks 3-4 reuse)
            for b in range(BLOCKS):
                qe = ((b + 1) * 128 - 1) // QW
                while qdone < qe:
                    qdone += 1
                    pe.wait_ge(s_exq[qdone], 32)
                if b >= 4:
                    pe.wait_ge(s_ehcp, (b - 4) // 4 + 1)
                for t in range(2):
                    mm = nc.tensor.matmul(
                        eh_slot(b),
                        exT[t][:, 128 * b : 128 * (b + 1)],
                        ebf[t][:, 0:D],
                        start=(t == 0),
                        stop=(t == 1),
                    )
                    if t == 1:
                        mm.then_inc(s_ehmm, 1)
            # main attention loop
            pe.wait_ge(s_excp, NCH)
            pe.wait_ge(s_ehcp, (BLOCKS + 3) // 4)   # eh drains done
            for s in range(NS):
                pe.wait_ge(s_maskG, 8 * (s + 1))
                pe.wait_ge(s_maskD, 8 * (s + 1))
                for i in range(BPS):
                    b = s * BPS + i
                    slot = ps_att[:, 512 * i : 512 * i + 258]
                    if b >= BPS:
                        pe.wait_ge(s_stt, b - BPS + 1)
                    for j in range(8):
                        mm = nc.tensor.matmul(
                            slot,
                            pmv(s, j)[:, 128 * i : 128 * (i + 1)],
                            kcwhE[j],
                            start=(j == 0),
                            stop=(j == 7),
                        )
                        if j == 7:
                            mm.then_inc(s_attmm, 1)

        # ---------------- ACT ----------------
        @block.scalar
        def _(act):
            act.wait_ge(s_kcj, 8)
            nc.scalar.activation(
                drow[:, 0:8], kc_score[:, 0:8], AF.Exp, scale=ALPHA
            ).then_inc(s_dj, 1)
            for h in range(2):
                kw = HALVES[h][1] // 128
                act.wait_ge(s_crl[h], 16)
                nc.scalar.activation(
                    crowE[:, 0:kw], crow128[:, 0:kw], AF.Exp, scale=ALPHA
                ).then_inc(s_cre, 1)

            def elu_exp(g):
                gw = 512 if 2 * g + 1 < BLOCKS else 256
                act.wait_ge(s_stt, min(2 * g + 2, BLOCKS))
                if g >= 4:
                    act.wait_ge(s_mb, g - 3)
                nc.scalar.activation(
                    ebuf[:, (g % 4) * 512 : (g % 4) * 512 + gw],
                    zb[:, (g % 4) * 512 : (g % 4) * 512 + gw],
                    AF.Exp,
                ).then_inc(s_eluE, 1)

            g_done = 0
            n_a = 0
            for s in range(NS):
                h = 0 if s < 4 else 1
                for j in range(8):
                    k = IDX[(s, j)]
                    act.wait_ge(s_exb[h], 16)
                    act.wait_ge(s_kcj, j + 1)
                    if s >= 2:
                        act.wait_ge(s_attmm, BPS * (s - 1))
                    src = exb[:, s * W : (s + 1) * W]
                    if (s, j) in A_ITEMS:
                        if n_a >= 1:
                            act.wait_ge(s_pmA, n_a)   # ltb WAR vs prior A Exp
                        nc.scalar.activation(
                            ltb[:, :], src, AF.Prelu,
                            bias=kc_score[:, j : j + 1], scale=1.0, alpha=ALPHA,
                        ).then_inc(s_lt, 1)
                        n_a += 1
                        act.wait_ge(s_lt, n_a)
                        nc.scalar.activation(
                            pmv(s, j)[:, :], ltb[:, :], AF.Exp
                        ).then_inc(s_pmA, 1)
                    else:
                        nc.scalar.activation(
                            pmv(s, j)[:, :], src, AF.Exp,
                            bias=kc_score[:, j : j + 1], scale=1.0,
                        ).then_inc(s_t2, 1)
                if s >= 1:
                    g_hi = (BPS * s - 2) // 2
                    while g_done <= g_hi:
                        elu_exp(g_done)
                        g_done += 1
            while g_done < NG:
                elu_exp(g_done)
                g_done += 1

        # ---------------- DVE ----------------
        @block.vector
        def _(dv):
            dv.wait_ge(s_w1a1, 1)
            nc.vector.tensor_copy(out=w1a1c[:, :], in_=ps_w1a1).then_inc(s_w1a1c, 1)
            dv.wait_ge(s_a2b, 16)
            for j in range(8):
                dv.wait_ge(s_kcwh, j + 1)
                if j >= 1:
                    dv.wait_ge(s_kcj, j)   # serialize kcs_tmp WAW
                nc.vector.tensor_copy(out=kcwhE[j][:, 0:D], in_=ps_kcwh[j % 2])
                nc.vector.memset(kcwhE[j][:, D : D + 1], 1.0)
                nc.vector.memset(kcwhE[j][:, D + 1 : D + 2], 0.0)
                nc.vector.scalar_tensor_tensor(
                    out=kcs_tmp[:, :],
                    in0=ps_kcwh[j % 2],
                    scalar=1.0,
                    in1=a2b[:, :],
                    op0=ALU.mult,
                    op1=ALU.mult,
                    accum_out=kc_score[:, j : j + 1],
                ).then_inc(s_kcj, 1)
            for c in range(NCH):
                dv.wait_ge(s_exsc, c + 1)
                if c >= 2:
                    dv.wait_ge(s_bnc[c % 2], 16 * (c // 2))
                nc.vector.tensor_copy(
                    out=excp[0:1, (c % 2) * CHUNK : (c % 2) * CHUNK + CHUNK],
                    in_=ps_exsc[c % 3],
                ).then_inc(s_excp, 1)

            eh_next = [0]

            def drain_eh(nbatches):
                # batches of 4 blocks; blocks 4m..4m+3 live in banks 3..6
                while eh_next[0] < min(nbatches, (BLOCKS + 3) // 4):
                    m = eh_next[0]
                    hi = min(4 * m + 4, BLOCKS)
                    nb4 = hi - 4 * m
                    dv.wait_ge(s_ehmm, hi)
                    nc.vector.tensor_copy(
                        out=ehb[:, D * 4 * m : D * (4 * m + nb4)],
                        in_=ps_att[:, 1536 : 1536 + 512 * nb4].rearrange(
                            "p (b c) -> p b c", c=512
                        )[:, :, 0:D],
                    ).then_inc(s_ehcp, 1)
                    eh_next[0] += 1

            dv.wait_ge(s_dj, 1)
            pend_fin = []
            n_fin = [0]   # s_fin incs
            n_rc = [0]    # s_rc incs (gathers)
            deferred = [None]   # deferred mask-tail closure

            def flush_tail():
                if deferred[0] is not None:
                    fn = deferred[0]
                    deferred[0] = None
                    fn()

            def drain_fins():
                # software-pipelined pairs: all F (ts) first, then all maxes
                gs = list(pend_fin)
                pend_fin.clear()
                for g in gs:
                    gw = 512 if 2 * g + 1 < BLOCKS else 256
                    dv.wait_ge(s_eluE, g + 1)
                    nc.vector.tensor_scalar(
                        out=ebuf[:, (g % 4) * 512 : (g % 4) * 512 + gw],
                        in0=ebuf[:, (g % 4) * 512 : (g % 4) * 512 + gw],
                        scalar1=1.0,
                        scalar2=-1.0,
                        op0=ALU.min,
                        op1=ALU.add,
                    ).then_inc(s_fin, 1)
                    n_fin[0] += 1
                for g in gs:
                    gw = 512 if 2 * g + 1 < BLOCKS else 256
                    dv.wait_ge(s_fin, n_fin[0] - len(gs) + gs.index(g) + 1)
                    nc.vector.tensor_tensor(
                        out=mb[:, (g % 4) * 512 : (g % 4) * 512 + gw],
                        in0=zb[:, (g % 4) * 512 : (g % 4) * 512 + gw],
                        in1=ebuf[:, (g % 4) * 512 : (g % 4) * 512 + gw],
                        op=ALU.max,
                    ).then_inc(s_mb, 1)

            def do_stt(b):
                s2, i2 = divmod(b, BPS)
                g, qq = divmod(b, 2)
                if g >= 4:
                    dv.wait_ge(s_st[g % 4], 16 * (g // 4))   # zb slot reuse
                dv.wait_ge(s_ehcp, b // 4 + 1)   # ehb[b] drained
                dv.wait_ge(s_rc, s2 + 1)     # recip of stripe s2 retired
                nc.vector.scalar_tensor_tensor(
                    out=zb[:, (g % 4) * 512 + 256 * qq : (g % 4) * 512 + 256 * qq + 256],
                    in0=ps_att[:, 512 * i2 : 512 * i2 + 256],
                    scalar=recipb[:, (s2 % 2) * BPS + i2 : (s2 % 2) * BPS + i2 + 1],
                    in1=ehb[:, D * b : D * (b + 1)],
                    op0=ALU.mult,
                    op1=ALU.mult,
                ).then_inc(s_stt, 1)
                if qq == 1 or b == BLOCKS - 1:
                    pend_fin.append(g)

            def gather_recip(s2):
                dv.wait_ge(s_attmm, BPS * (s2 + 1))
                if s2 >= 2:
                    dv.wait_ge(s_stt, BPS * (s2 - 1))   # recipb slot WAR
                nc.vector.reciprocal(
                    recipb[:, (s2 % 2) * BPS : (s2 % 2) * BPS + BPS],
                    ps_att[:, 256 : BPS * 512 : 512],
                ).then_inc(s_rc, 1)
                n_rc[0] += 1

            for s in range(NS):
                stt_q = list(range(BPS * (s - 1), BPS * s)) if s >= 1 else []
                for j in range(8):
                    k = IDX[(s, j)]
                    if s <= 1:
                        drain_eh(k + 1)   # pace eh drains through early items
                    if s >= 1 and j == 3:
                        drain_eh((BLOCKS + 3) // 4)   # PE att gate needs all
                        drain_fins()
                        gather_recip(s - 1)
                    if (s, j) in A_ITEMS:
                        flush_tail()
                        dv.wait_ge(s_pmA, NAC[k])
                    else:
                        dv.wait_ge(s_t2, BORD[(s, j)])
                        dv.wait_ge(s_cb[0 if s < 4 else 1], 16)
                        nc.vector.scalar_tensor_tensor(
                            out=pmv(s, j)[:, :],
                            in0=cb[:, s * W : (s + 1) * W],
                            scalar=drow[:, j : j + 1],
                            in1=pmv(s, j)[:, :],
                            op0=ALU.mult,
                            op1=ALU.max,
                        ).then_inc(s_pmB, 1)
                        flush_tail()

                    def mk_tail(s=s, j=j, k=k):
                        def fn():
                            if (s, j) in A_ITEMS:
                                dv.wait_ge(s_pmA, NAC[k])
                            else:
                                dv.wait_ge(s_pmB, NBC[k])
                            dv.wait_ge(s_adjt[j], 16 if s < 4 else 32)
                            lo = adj_col(s)
                            nc.vector.tensor_tensor(
                                out=pmv(s, j)[:, MASK_GPS:W],
                                in0=pmv(s, j)[:, MASK_GPS:W],
                                in1=adjb[j][:, lo + MASK_GPS : lo + W],
                                op=ALU.mult,
                            ).then_inc(s_maskD, 1)
                        return fn

                    deferred[0] = mk_tail()
                    if j >= 3 and stt_q:
                        do_stt(stt_q.pop(0))
                flush_tail()
                while stt_q:
                    do_stt(stt_q.pop(0))
            drain_fins()
            gather_recip(NS - 1)
            for b in range(BPS * (NS - 1), BLOCKS):
                do_stt(b)
                drain_fins()

    return nc


def _prep_shards(exercise_h, kc_h, adj_exercise_kc, W1, E, a):
    import ml_dtypes

    bf16 = ml_dtypes.bfloat16
    exercise_h = np.asarray(exercise_h, dtype=np.float32)
    kc_h = np.asarray(kc_h, dtype=np.float32)
    adj = np.asarray(adj_exercise_kc, dtype=np.int8)
    W1 = np.asarray(W1, dtype=np.float32)
    E = np.asarray(E, dtype=np.float32)
    a = np.asarray(a, dtype=np.float32)

    wpack = np.zeros((D, WPK), dtype=np.float32)
    wpack[:, 0:D] = W1
    wpack[:, D : 2 * D] = W1.T
    wpack[:, 2 * D : 2 * D + N_KC] = kc_h.T
    wpack[:, 1536] = a[:D, 0]
    wpack[0, 1537 : 1537 + D] = a[D:, 0]
    wpack = np.ascontiguousarray(wpack.astype(bf16))
    eM = np.ascontiguousarray(E.astype(bf16))

    in_maps = []
    for i in range(N_CORES):
        lo = i * SHARD
        exT = np.zeros((D, PAD), dtype=bf16)
        exT[:, :SHARD] = exercise_h[lo : lo + SHARD].T.astype(bf16)
        adjT = np.zeros((N_KC, PAD), dtype=np.int8)
        adjT[:, :SHARD] = adj[lo : lo + SHARD].T
        adjT[0, SHARD:] = 1
        in_maps.append(
            {
                "exT": np.ascontiguousarray(exT),
                "adjT": np.ascontiguousarray(adjT),
                "wpack": wpack,
                "eMat": eM,
            }
        )
    return in_maps


def kernel(exercise_h, kc_h, adj_exercise_kc, W1, E, a, _trace=False, _tmpdir=None):
    from concourse.bass_utils import run_bass_kernel_spmd

    if "nc" not in _CACHE:
        _CACHE["nc"] = _build_nc()
    nc = _CACHE["nc"]

    in_maps = _prep_shards(exercise_h, kc_h, adj_exercise_kc, W1, E, a)
    res = run_bass_kernel_spmd(
        nc, in_maps, list(range(N_CORES)), trace=_trace, tmpdir=_tmpdir
    )
    _CACHE["last_result"] = res
    out = np.concatenate(
        [
            np.asarray(res.results[i]["out"])[:SHARD].astype(np.float32)
            for i in range(N_CORES)
        ],
        axis=0,
    )
    return out


# revision 20
# speedup vs baseline: 1.3585x; 1.3585x over previous
"""Trainium2 Bass kernel for GAT-style exercise->KC message passing (v4).

Math (per reference):
  kc_Wh = kc_h @ W1
  z[i,j] = ex_score[i] + kc_score[j]
  p[i,j] = adj * exp(leaky(z));  leaky via Prelu on ACT (A-items) or
           max(exp(z), C_i*D_j) with C=exp(0.2 ex), D=exp(0.2 kc) (B-items)
  attn   = p / rowsum(p)   (rowsum via ones column in the attention matmul)
  out    = elu((attn @ kc_Wh) * (exercise_h @ E)), elu(x)=max(x, min(exp(x),1)-1)

Layout: KC on partitions (8 j-chunks of 128), EX on the free axis in 7
stripes of 896 cols. Per stripe: 8 items (s,j) produce masked pm tiles;
6 items/stripe run on ACT (Prelu+Exp, software-pipelined pairs), 2 on DVE
(single fused scalar_tensor_tensor). Masks are full-width, split by j:
js 0-4 on GPSIMD, js 5-7 on DVE. PE runs the attention matmuls per
stripe; the softmax/ELU epilogue is stripe-granular and runs at the END
of the next stripe's DVE pass so PE overlaps DVE. Eh = exercise_h @ E is
computed early by PE (rotating through ps_att banks 3-6) and drained to
SBUF bf16 in batches of 4 blocks.

All DRAM I/O bf16/int8; psum fp32. Exercise rows sharded 8 ways.
DMA-completion semaphores can increment out of order across queues, so
every DMA wait is against the full count of a dedicated semaphore (or a
parity-split one where issue-order gating bounds the contributors).
Same-engine dependent ops carry explicit semaphore edges (engine queues
overlap reads with in-flight writes).
"""

import sys

sys.path.insert(0, "/opt/trn_rl_repo")

import numpy as np

N_CORES = 8
N_EX = 50000
N_KC = 1024
D = 256
SHARD = N_EX // N_CORES          # 6250
PAD = 6272                       # 49 * 128
BLOCKS = PAD // 128              # 49
NS = 7                           # stripes
W = PAD // NS                    # 896
BPS = W // 128                   # 7 blocks per stripe
ALPHA = 0.2
WPK = 1808
CHUNK = 448
NCH = PAD // CHUNK               # 14
QW = PAD // 4                    # 1568 exT load quarter
HALVES = [(0, 3584), (3584, 2688)]   # stripe-aligned halves (0-3 / 4-6)
B_JS = (5, 6)                    # items on the DVE fused path
GPS_JS = (0, 1, 2, 3, 4)         # mask js on gpsimd (full width)
DVE_JS = (5, 6, 7)               # mask js on DVE (full width)

_CACHE = {}


def _build_nc(sim_safe=False):
    import concourse.bass as bass
    import concourse.mybir as mybir

    f32 = mybir.dt.float32
    bf16 = mybir.dt.bfloat16
    i8 = mybir.dt.int8
    AF = mybir.ActivationFunctionType
    ALU = mybir.AluOpType
    AF_LEAKY = AF.Relu if sim_safe else AF.Prelu

    nc = bass.Bass()

    exT_d = nc.declare_dram_parameter("exT", [D, PAD], bf16, isOutput=False)
    adj_d = nc.declare_dram_parameter("adjT", [N_KC, PAD], i8, isOutput=False)
    wpk_d = nc.declare_dram_parameter("wpack", [D, WPK], bf16, isOutput=False)
    e_d = nc.declare_dram_parameter("eMat", [D, D], bf16, isOutput=False)
    out_d = nc.declare_dram_parameter("out", [PAD, D], bf16, isOutput=True)
    exrow_s = nc.dram_tensor("exrow_s", [1, PAD], bf16)
    crow_s = nc.dram_tensor("crow_s", [1, PAD], bf16)

    from contextlib import ExitStack

    es = ExitStack()
    _ctr = [0]

    def _nm(pfx):
        _ctr[0] += 1
        return f"{pfx}{_ctr[0]}"

    sb = lambda shape, dt: es.enter_context(nc.sbuf_tensor(_nm("t"), shape, dt))
    ps = lambda shape, dt: es.enter_context(nc.psum_tensor(_nm("p"), shape, dt))
    sem = lambda: es.enter_context(nc.semaphore(name=_nm("s")))

    ITEMS = [(s, j) for s in range(NS) for j in range(8)]
    IDX = {it: k for k, it in enumerate(ITEMS)}
    BORD, NAC = {}, []
    nb = na = 0
    for it in ITEMS:
        if it[1] in B_JS:
            nb += 1
            BORD[it] = nb
        else:
            na += 1
        NAC.append(na)

    ADJH = 3584

    def adj_col(s):
        return s * W if s < 4 else s * W - ADJH

    with es:
        # ---- SBUF ----
        wp0 = sb([128, WPK], bf16); wp1 = sb([128, WPK], bf16)
        ebf0 = sb([128, D], bf16); ebf1 = sb([128, D], bf16)
        a2b = sb([128, D], bf16)
        w1a1c = sb([128, 2], bf16)
        kcwhE_all = sb([128, 8 * 264], bf16)
        kc_score = sb([128, 8], f32)
        kcs_tmp = sb([128, D], f32)
        drow = sb([128, 8], f32)
        adj_all = sb([128, 8 * ADJH], bf16)
        exT0 = sb([128, PAD], bf16); exT1 = sb([128, PAD], bf16)
        exb = sb([128, PAD], bf16)
        cb = sb([128, PAD], bf16)
        pm2 = sb([128, 2 * 8 * W], bf16)
        ltb = sb([128, 2 * W], bf16)
        excp = sb([1, 2 * CHUNK], bf16)
        crow128 = sb([128, 28], bf16)
        crowE = sb([128, 28], bf16)
        ehb = sb([128, BLOCKS * D], bf16)
        recipb = sb([128, 2 * BPS], f32)
        EW = BPS * D   # 1792 epilogue cols per stripe
        zb = sb([128, 2 * EW], bf16)
        ebuf = sb([128, 2 * EW], bf16)
        mb = sb([128, 2 * EW], bf16)

        # ---- PSUM ----
        ps_att = ps([128, BPS * 512], f32)

        def eh_slot(b):
            lo = 512 * (3 + b % 4)
            return ps_att[:, lo : lo + D]

        ps_w1a1 = ps_att[:, 1472:1474]
        ps_exsc = [
            ps_att[0:1, 0:CHUNK],
            ps_att[0:1, 512 : 512 + CHUNK],
            ps_att[0:1, 1024 : 1024 + CHUNK],
        ]
        ps_kcwh = [ps_att[:, 1536:1792], ps_att[:, 2048:2304]]

        wp = [wp0, wp1]
        exT = [exT0, exT1]
        ebf = [ebf0, ebf1]
        w1 = [wp[t][:, 0:D] for t in range(2)]
        w1T = [wp[t][:, D : 2 * D] for t in range(2)]
        kchT = [wp[t][:, 2 * D : 2 * D + N_KC] for t in range(2)]
        a1col = [wp[t][:, 1536:1537] for t in range(2)]
        kcwhE = [kcwhE_all[:, 264 * j : 264 * j + 258] for j in range(8)]
        adjb = [adj_all[:, ADJH * j : ADJH * (j + 1)] for j in range(8)]

        def pmv(s, j):
            base = (s % 2) * 8 * W + j * W
            return pm2[:, base : base + W]

        s_wp = sem(); s_ebf = sem(); s_a2b = sem()
        s_exq = [sem() for _ in range(4)]
        s_adjt = [sem() for _ in range(8)]
        s_w1a1 = sem(); s_w1a1c = sem(); s_kcwh = sem(); s_kcj = sem()
        s_dj = sem(); s_exsc = sem(); s_excp = sem()
        s_bnc = [sem(), sem()]
        s_exb = [sem(), sem()]
        s_crl = [sem(), sem()]
        s_cre = sem()
        s_crs = [sem(), sem()]
        s_cb = [sem(), sem()]
        s_pmB = sem(); s_pmA = sem(); s_t2 = sem()
        s_maskG = sem(); s_maskD = sem()
        s_attmm = sem(); s_ehmm = sem(); s_ehcp = sem()
        s_stt = sem(); s_eluE = sem(); s_mb = sem()
        s_st = [sem(), sem()]
        s_fin = sem(); s_rc = sem(); s_lt = sem()

        block = es.enter_context(nc.Block())

        # helper: monotone wait pruning per engine
        def waiter(eng):
            seen = {}

            def wg(s, v):
                if seen.get(id(s), -1) < v:
                    seen[id(s)] = v
                    eng.wait_ge(s, v)

            return wg

        # ---------------- SYNC (SP) ----------------
        @block.sync
        def _(sync):
            sync.dma_start(out=wp0[:, :], in_=wpk_d[0:128, :]).then_inc(s_wp, 16)
            sync.dma_start(out=wp1[:, :], in_=wpk_d[128:256, :]).then_inc(s_wp, 16)
            sync.dma_start(out=ebf0[:, :], in_=e_d[0:128, :]).then_inc(s_ebf, 16)
            sync.dma_start(out=ebf1[:, :], in_=e_d[128:256, :]).then_inc(s_ebf, 16)
            sync.dma_start(
                out=a2b[:, :],
                in_=wpk_d[0:1, 1537 : 1537 + D].to_broadcast((128, D)),
            ).then_inc(s_a2b, 16)
            for q in range(4):
                for t in range(2):
                    sync.dma_start(
                        out=exT[t][:, q * QW : (q + 1) * QW],
                        in_=exT_d[128 * t : 128 * (t + 1), q * QW : (q + 1) * QW],
                    ).then_inc(s_exq[q], 16)
            for c in range(NCH):
                sync.wait_ge(s_excp, c + 1)
                sync.dma_start(
                    out=exrow_s[0:1, c * CHUNK : (c + 1) * CHUNK],
                    in_=excp[0:1, (c % 2) * CHUNK : (c % 2) * CHUNK + CHUNK],
                ).then_inc(s_bnc[c % 2], 16)
            for h in range(2):
                off, hw = HALVES[h]
                kw = hw // 128
                nche = (off + hw) // CHUNK
                sync.wait_ge(s_bnc[0], 16 * ((nche + 1) // 2))
                sync.wait_ge(s_bnc[1], 16 * (nche // 2))
                sync.dma_start(
                    out=exb[:, off : off + hw],
                    in_=exrow_s[0:1, off : off + hw].to_broadcast((128, hw)),
                ).then_inc(s_exb[h], 16)
                if h == 1:
                    sync.wait_ge(s_cre, 1)
                sync.dma_start(
                    out=crow128[:, 0:kw], in_=exrow_s[0:1, off : off + hw]
                ).then_inc(s_crl[h], 16)
                sync.wait_ge(s_cre, h + 1)
                sync.dma_start(
                    out=crow_s[0:1, off : off + hw], in_=crowE[:, 0:kw]
                ).then_inc(s_crs[h], 16)
                sync.wait_ge(s_crs[h], 16)
                sync.dma_start(
                    out=cb[:, off : off + hw],
                    in_=crow_s[0:1, off : off + hw].to_broadcast((128, hw)),
                ).then_inc(s_cb[h], 16)
            for s2 in range(NS):
                sync.wait_ge(s_mb, s2 + 1)
                sync.dma_start(
                    out=out_d[W * s2 : W * (s2 + 1), :].rearrange(
                        "(q p) c -> p q c", p=128
                    ),
                    in_=mb[:, (s2 % 2) * EW : (s2 % 2) * EW + EW],
                ).then_inc(s_st[s2 % 2], 16)
            sync.wait_ge(s_st[0], 16 * 4)
            sync.wait_ge(s_st[1], 16 * 3)

        # ---------------- GPSIMD: adj cast DMAs + masks (js 0-4) ----------------
        @block.gpsimd
        def _(gp):
            wg = waiter(gp)
            for j in range(8):
                gp.dma_start(
                    out=adjb[j][:, 0:ADJH],
                    in_=adj_d[128 * j : 128 * (j + 1), 0:ADJH],
                ).then_inc(s_adjt[j], 16)
            for s in range(NS):
                for j in GPS_JS:
                    k = IDX[(s, j)]
                    wg(s_pmA, NAC[k])
                    wg(s_adjt[j], 16 if s < 4 else 32)
                    lo = adj_col(s)
                    gp.tensor_tensor(
                        out=pmv(s, j)[:, :],
                        in0=pmv(s, j)[:, :],
                        in1=adjb[j][:, lo : lo + W],
                        op=ALU.mult,
                    ).then_inc(s_maskG, 1)
                    if s == 3:
                        wg(s_maskG, 5 * 3 + GPS_JS.index(j) + 1)  # own retired
                        gp.dma_start(
                            out=adjb[j][:, 0 : HALVES[1][1]],
                            in_=adj_d[
                                128 * j : 128 * (j + 1),
                                HALVES[1][0] : HALVES[1][0] + HALVES[1][1],
                            ],
                        ).then_inc(s_adjt[j], 16)
                if s == 3:
                    for ji, j in enumerate(DVE_JS):
                        wg(s_maskD, 3 * 3 + ji + 1)
                        gp.dma_start(
                            out=adjb[j][:, 0 : HALVES[1][1]],
                            in_=adj_d[
                                128 * j : 128 * (j + 1),
                                HALVES[1][0] : HALVES[1][0] + HALVES[1][1],
                            ],
                        ).then_inc(s_adjt[j], 16)

        # ---------------- PE ----------------
        @block.tensor
        def _(pe):
            pe.wait_ge(s_wp, 32)
            for t in range(2):
                for kt in range(2):
                    mm = nc.tensor.matmul(
                        ps_w1a1[:, t : t + 1],
                        w1T[kt][:, 128 * t : 128 * (t + 1)],
                        a1col[kt],
                        start=(kt == 0),
                        stop=(kt == 1),
                    )
                    if t == 1 and kt == 1:
                        mm.then_inc(s_w1a1, 1)
            for j in range(8):
                if j >= 2:
                    pe.wait_ge(s_kcj, j - 1)
                for t in range(2):
                    mm = nc.tensor.matmul(
                        ps_kcwh[j % 2],
                        kchT[t][:, 128 * j : 128 * (j + 1)],
                        w1[t],
                        start=(t == 0),
                        stop=(t == 1),
                    )
                    if t == 1:
                        mm.then_inc(s_kcwh, 1)
            pe.wait_ge(s_w1a1c, 1)
            qdone = -1
            for cix in range(NCH):
                qe = ((cix + 1) * CHUNK - 1) // QW
                while qdone < qe:
                    qdone += 1
                    pe.wait_ge(s_exq[qdone], 32)
                if cix >= 3:
                    pe.wait_ge(s_excp, cix - 2)
                for t in range(2):
                    mm = nc.tensor.matmul(
                        ps_exsc[cix % 3],
                        w1a1c[:, t : t + 1],
                        exT[t][:, cix * CHUNK : (cix + 1) * CHUNK],
                        start=(t == 0),
                        stop=(t == 1),
                    )
                    if t == 1:
                        mm.then_inc(s_exsc, 1)
            # Eh early, rotating through ps_att banks 3-6; drains in 4-batches
            pe.wait_ge(s_ebf, 32)
            pe.wait_ge(s_kcj, 8)
            for b in range(BLOCKS):
                qe = ((b + 1) * 128 - 1) // QW
                while qdone < qe:
                    qdone += 1
                    pe.wait_ge(s_exq[qdone], 32)
                if b >= 4:
                    pe.wait_ge(s_ehcp, (b - 4) // 4 + 1)
                for t in range(2):
                    mm = nc.tensor.matmul(
                        eh_slot(b),
                        exT[t][:, 128 * b : 128 * (b + 1)],
                        ebf[t][:, 0:D],
                        start=(t == 0),
                        stop=(t == 1),
                    )
                    if t == 1:
                        mm.then_inc(s_ehmm, 1)
            # main attention loop
            pe.wait_ge(s_excp, NCH)
            pe.wait_ge(s_ehcp, (BLOCKS + 3) // 4)
            for s in range(NS):
                pe.wait_ge(s_maskG, len(GPS_JS) * (s + 1))
                pe.wait_ge(s_maskD, len(DVE_JS) * (s + 1))
                for i in range(BPS):
                    b = s * BPS + i
                    slot = ps_att[:, 512 * i : 512 * i + 258]
                    if b >= BPS:
                        pe.wait_ge(s_stt, b - BPS + 1)
                    for j in range(8):
                        mm = nc.tensor.matmul(
                            slot,
                            pmv(s, j)[:, 128 * i : 128 * (i + 1)],
                            kcwhE[j],
                            start=(j == 0),
                            stop=(j == 7),
                        )
                        if j == 7:
                            mm.then_inc(s_attmm, 1)

        # ---------------- ACT ----------------
        @block.scalar
        def _(act):
            wg = waiter(act)
            act.wait_ge(s_kcj, 8)
            nc.scalar.activation(
                drow[:, 0:8], kc_score[:, 0:8], AF.Exp, scale=ALPHA
            ).then_inc(s_dj, 1)
            for h in range(2):
                kw = HALVES[h][1] // 128
                act.wait_ge(s_crl[h], 16)
                nc.scalar.activation(
                    crowE[:, 0:kw], crow128[:, 0:kw], AF.Exp, scale=ALPHA
                ).then_inc(s_cre, 1)

            n_a = [0]
            pend = [None]

            def flush_exp():
                if pend[0] is not None:
                    sj, slot = pend[0]
                    pend[0] = None
                    act.wait_ge(s_lt, n_a[0])
                    nc.scalar.activation(
                        pmv(*sj)[:, :], ltb[:, slot * W : slot * W + W], AF.Exp
                    ).then_inc(s_pmA, 1)

            def elu_exp(s2):
                wg(s_stt, BPS * (s2 + 1))
                if s2 >= 2:
                    wg(s_mb, s2 - 1)   # ebuf slot reuse
                nc.scalar.activation(
                    ebuf[:, (s2 % 2) * EW : (s2 % 2) * EW + EW],
                    zb[:, (s2 % 2) * EW : (s2 % 2) * EW + EW],
                    AF.Exp,
                ).then_inc(s_eluE, 1)

            for s in range(NS):
                h = 0 if s < 4 else 1
                wg(s_exb[h], 16)
                for j in range(8):
                    k = IDX[(s, j)]
                    wg(s_kcj, j + 1)
                    if s >= 2:
                        wg(s_attmm, BPS * (s - 1))   # pm slot reuse
                    src = exb[:, s * W : (s + 1) * W]
                    if j in B_JS:
                        flush_exp()
                        nc.scalar.activation(
                            pmv(s, j)[:, :], src, AF.Exp,
                            bias=kc_score[:, j : j + 1], scale=1.0,
                        ).then_inc(s_t2, 1)
                    else:
                        slot = n_a[0] % 2
                        if n_a[0] >= 2:
                            wg(s_pmA, n_a[0] - 1)   # ltb slot WAR
                        nc.scalar.activation(
                            ltb[:, slot * W : slot * W + W], src, AF_LEAKY,
                            bias=kc_score[:, j : j + 1], scale=1.0, alpha=ALPHA,
                        ).then_inc(s_lt, 1)
                        n_a[0] += 1
                        flush_exp()
                        pend[0] = ((s, j), slot)
                flush_exp()
                if s >= 2:
                    elu_exp(s - 2)
            elu_exp(NS - 2)
            elu_exp(NS - 1)

        # ---------------- DVE ----------------
        @block.vector
        def _(dv):
            wg = waiter(dv)
            dv.wait_ge(s_w1a1, 1)
            nc.vector.tensor_copy(out=w1a1c[:, :], in_=ps_w1a1).then_inc(s_w1a1c, 1)
            dv.wait_ge(s_a2b, 16)
            ones_ap = kcwhE_all[:, :].rearrange("p (j c) -> p j c", c=264)[:, :, 256:257]
            zero_ap = kcwhE_all[:, :].rearrange("p (j c) -> p j c", c=264)[:, :, 257:258]
            nc.vector.memset(ones_ap, 1.0)
            nc.vector.memset(zero_ap, 0.0)
            for j in range(8):
                dv.wait_ge(s_kcwh, j + 1)
                if j >= 1:
                    dv.wait_ge(s_kcj, j)   # kcs_tmp WAW serialization
                nc.vector.tensor_copy(out=kcwhE[j][:, 0:D], in_=ps_kcwh[j % 2])
                nc.vector.scalar_tensor_tensor(
                    out=kcs_tmp[:, :],
                    in0=ps_kcwh[j % 2],
                    scalar=1.0,
                    in1=a2b[:, :],
                    op0=ALU.mult,
                    op1=ALU.mult,
                    accum_out=kc_score[:, j : j + 1],
                ).then_inc(s_kcj, 1)
            for c in range(NCH):
                dv.wait_ge(s_exsc, c + 1)
                if c >= 2:
                    dv.wait_ge(s_bnc[c % 2], 16 * (c // 2))
                nc.vector.tensor_copy(
                    out=excp[0:1, (c % 2) * CHUNK : (c % 2) * CHUNK + CHUNK],
                    in_=ps_exsc[c % 3],
                ).then_inc(s_excp, 1)

            eh_next = [0]
            NEHB = (BLOCKS + 3) // 4

            def drain_eh(nbatches):
                while eh_next[0] < min(nbatches, NEHB):
                    m = eh_next[0]
                    hi = min(4 * m + 4, BLOCKS)
                    nb4 = hi - 4 * m
                    dv.wait_ge(s_ehmm, hi)
                    nc.vector.tensor_copy(
                        out=ehb[:, D * 4 * m : D * (4 * m + nb4)],
                        in_=ps_att[:, 1536 : 1536 + 512 * nb4].rearrange(
                            "p (b c) -> p b c", c=512
                        )[:, :, 0:D],
                    ).then_inc(s_ehcp, 1)
                    eh_next[0] += 1

            dv.wait_ge(s_dj, 1)
            nB = [0]
            nD = [0]

            def epilogue(s2):
                # gather+recip, 7 stts into zb stripe slot
                dv.wait_ge(s_attmm, BPS * (s2 + 1))
                if s2 >= 2:
                    wg(s_stt, BPS * (s2 - 1))   # recipb slot WAR
                nc.vector.reciprocal(
                    recipb[:, (s2 % 2) * BPS : (s2 % 2) * BPS + BPS],
                    ps_att[:, 256 : BPS * 512 : 512],
                ).then_inc(s_rc, 1)
                wg(s_rc, s2 + 1)
                if s2 >= 2:
                    wg(s_st[s2 % 2], 16 * (s2 // 2))   # zb slot reuse
                wg(s_ehcp, NEHB)
                for i2 in range(BPS):
                    b = s2 * BPS + i2
                    nc.vector.scalar_tensor_tensor(
                        out=zb[:, (s2 % 2) * EW + 256 * i2 : (s2 % 2) * EW + 256 * i2 + 256],
                        in0=ps_att[:, 512 * i2 : 512 * i2 + 256],
                        scalar=recipb[:, (s2 % 2) * BPS + i2 : (s2 % 2) * BPS + i2 + 1],
                        in1=ehb[:, D * b : D * (b + 1)],
                        op0=ALU.mult,
                        op1=ALU.mult,
                    ).then_inc(s_stt, 1)

            def fin(s2):
                # F = min(E,1)-1 (in place) ; mb = max(zb, F)
                wg(s_eluE, s2 + 1)
                if s2 >= 2:
                    wg(s_st[s2 % 2], 16 * (s2 // 2))   # mb slot reuse
                nc.vector.tensor_scalar(
                    out=ebuf[:, (s2 % 2) * EW : (s2 % 2) * EW + EW],
                    in0=ebuf[:, (s2 % 2) * EW : (s2 % 2) * EW + EW],
                    scalar1=1.0,
                    scalar2=-1.0,
                    op0=ALU.min,
                    op1=ALU.add,
                ).then_inc(s_fin, 1)
                wg(s_fin, s2 + 1)
                nc.vector.tensor_tensor(
                    out=mb[:, (s2 % 2) * EW : (s2 % 2) * EW + EW],
                    in0=zb[:, (s2 % 2) * EW : (s2 % 2) * EW + EW],
                    in1=ebuf[:, (s2 % 2) * EW : (s2 % 2) * EW + EW],
                    op=ALU.max,
                ).then_inc(s_mb, 1)

            for s in range(NS):
                h = 0 if s < 4 else 1
                if s == 0:
                    drain_eh(4)
                elif s == 1:
                    drain_eh(NEHB)
                # fused stt for B-items
                for j in B_JS:
                    dv.wait_ge(s_t2, BORD[(s, j)])
                    wg(s_cb[h], 16)
                    nc.vector.scalar_tensor_tensor(
                        out=pmv(s, j)[:, :],
                        in0=cb[:, s * W : (s + 1) * W],
                        scalar=drow[:, j : j + 1],
                        in1=pmv(s, j)[:, :],
                        op0=ALU.mult,
                        op1=ALU.max,
                    ).then_inc(s_pmB, 1)
                    nB[0] += 1
                # masks js 5,6,7 (full width)
                for j in DVE_JS:
                    k = IDX[(s, j)]
                    if j in B_JS:
                        wg(s_pmB, nB[0])
                    else:
                        dv.wait_ge(s_pmA, NAC[k])
                    wg(s_adjt[j], 16 if s < 4 else 32)
                    lo = adj_col(s)
                    nc.vector.tensor_tensor(
                        out=pmv(s, j)[:, :],
                        in0=pmv(s, j)[:, :],
                        in1=adjb[j][:, lo : lo + W],
                        op=ALU.mult,
                    ).then_inc(s_maskD, 1)
                    nD[0] += 1
                if s >= 1:
                    epilogue(s - 1)
                if s >= 2:
                    fin(s - 2)
            epilogue(NS - 1)
            fin(NS - 2)
            fin(NS - 1)

    return nc


def _prep_shards(exercise_h, kc_h, adj_exercise_kc, W1, E, a):
    import ml_dtypes

    bf16 = ml_dtypes.bfloat16
    exercise_h = np.asarray(exercise_h, dtype=np.float32)
    kc_h = np.asarray(kc_h, dtype=np.float32)
    adj = np.asarray(adj_exercise_kc, dtype=np.int8)
    W1 = np.asarray(W1, dtype=np.float32)
    E = np.asarray(E, dtype=np.float32)
    a = np.asarray(a, dtype=np.float32)

    wpack = np.zeros((D, WPK), dtype=np.float32)
    wpack[:, 0:D] = W1
    wpack[:, D : 2 * D] = W1.T
    wpack[:, 2 * D : 2 * D + N_KC] = kc_h.T
    wpack[:, 1536] = a[:D, 0]
    wpack[0, 1537 : 1537 + D] = a[D:, 0]
    wpack = np.ascontiguousarray(wpack.astype(bf16))
    eM = np.ascontiguousarray(E.astype(bf16))

    in_maps = []
    for i in range(N_CORES):
        lo = i * SHARD
        exT = np.zeros((D, PAD), dtype=bf16)
        exT[:, :SHARD] = exercise_h[lo : lo + SHARD].T.astype(bf16)
        adjT = np.zeros((N_KC, PAD), dtype=np.int8)
        adjT[:, :SHARD] = adj[lo : lo + SHARD].T
        adjT[0, SHARD:] = 1
        in_maps.append(
            {
                "exT": np.ascontiguousarray(exT),
                "adjT": np.ascontiguousarray(adjT),
                "wpack": wpack,
                "eMat": eM,
            }
        )
    return in_maps


def kernel(exercise_h, kc_h, adj_exercise_kc, W1, E, a, _trace=False, _tmpdir=None):
    from concourse.bass_utils import run_bass_kernel_spmd

    if "nc" not in _CACHE:
        _CACHE["nc"] = _build_nc()
    nc = _CACHE["nc"]

    in_maps = _prep_shards(exercise_h, kc_h, adj_exercise_kc, W1, E, a)
    res = run_bass_kernel_spmd(
        nc, in_maps, list(range(N_CORES)), trace=_trace, tmpdir=_tmpdir
    )
    _CACHE["last_result"] = res
    out = np.concatenate(
        [
            np.asarray(res.results[i]["out"])[:SHARD].astype(np.float32)
            for i in range(N_CORES)
        ],
        axis=0,
    )
    return out
